# revision 1
# baseline (speedup 1.0000x reference)
"""ExpertsChooseMlp Trainium2 kernel.

Full inputs in, full output out. Sharding: 8 cores = 4 batches x 2 expert-pairs.
Core m handles batch b=m//2 and experts {2g, 2g+1}, g=m%2. Each core computes
pout[T,O] = sum_{e in pair} combine[b,:,e,:] @ mlp_e(dispatch[b,:,e,:]^T @ x[b]);
the host sums the two partials per batch and adds b2.

Precision: x/w1/w2/h in bf16, dispatch_mask/combine_array/y in fp8-e4m3, all
accumulation in fp32 PSUM (measured 4e-3 max relative error vs the fp32
reference). The combine contraction runs as fp8 DoubleRow matmuls (2 C-planes
per pass, ~1.8x bf16 throughput). Layouts are chosen so the natural
(host-prepared) operand orientations feed the PE with zero on-device
transposes:
  xdT[D,C] = matmul(lhsT=x[b][T,D],  rhs=dm_e[T,C])     (K=T)
  hT[HE,C] = matmul(lhsT=w1_e[D,HE], rhs=xdT[D,C])      (K=D), then GELU+b1
  y[C,O]   = matmul(lhsT=hT[HE,C],   rhs=w2_e[HE,O])    (K=HE)
  pout[T,O]= matmul(lhsT=cmT_e[C,T], rhs=y[C,O])        (K=C, accum over e,
                                                         fp8 DoubleRow)
Only cmT (combine slice transposed) is materialized host-side.
Measured: ~128us HW exec per core (all 8 cores balanced, PE dense with zero
>200ns gaps between matmuls; remaining overhead is engine preamble + Tile
exit barrier).
"""
import sys

sys.path.insert(0, "/opt/trn_rl_repo")

import numpy as np
import ml_dtypes

import concourse.bacc as bacc
import concourse.mybir as mybir
import concourse.tile as tile
from concourse import bass_utils

B, T, D, E, C, HE, O = 4, 2048, 512, 4, 1024, 512, 512
P = 128
nKT = T // P      # 16 T-chunks
nMD = D // P      # 4  D-chunks
nMH = HE // P     # 4  HE-chunks
nKD = D // P      # 4
nCC = C // P      # 8  C-chunks
nKH = HE // P     # 4
nMT = T // P      # 16
NF = 512          # matmul free dim (one PSUM bank)

F32 = mybir.dt.float32
BF16 = mybir.dt.bfloat16
F8 = mybir.dt.float8e4
GELU = mybir.ActivationFunctionType.Gelu
DR = mybir.MatmulPerfMode.DoubleRow
nCP = nCC // 2    # 4  C pair-chunks (DoubleRow: K=256 per matmul)

_NC = None


def _build():
    nc = bacc.Bacc("TRN2", target_bir_lowering=False, debug=False,
                   enable_asserts=False, num_devices=1)
    xb = nc.dram_tensor("xb", [T, D], BF16, kind="ExternalInput").ap()
    dm = nc.dram_tensor("dm", [2, T, C], F8, kind="ExternalInput").ap()
    cmt = nc.dram_tensor("cmt", [2, C, T], F8, kind="ExternalInput").ap()
    w1 = nc.dram_tensor("w1", [2, D, HE], BF16, kind="ExternalInput").ap()
    w2 = nc.dram_tensor("w2", [2, HE, O], BF16, kind="ExternalInput").ap()
    b1 = nc.dram_tensor("b1s", [2, HE], F32, kind="ExternalInput").ap()
    pout = nc.dram_tensor("pout", [T, O], F32, kind="ExternalOutput").ap()

    with tile.TileContext(nc) as tc:
        with (
            tc.tile_pool(name="const", bufs=1) as const,
            tc.tile_pool(name="dmp", bufs=32) as dmp,
            tc.tile_pool(name="cmp", bufs=8) as cmp_,
            tc.tile_pool(name="inter", bufs=1) as inter,
            tc.tile_pool(name="yp", bufs=2) as yp,
            tc.tile_pool(name="outp", bufs=2) as outp,
            tc.tile_pool(name="psum", bufs=8, space="PSUM") as psp,
        ):
            # ---- resident constants (ACT HWDGE ring) ----
            # x split per-chunk so the first matmul isn't gated on a 2MB DMA
            x_sb = const.tile([P, nKT, D], BF16)
            for kt in range(nKT):
                nc.scalar.dma_start(x_sb[:, kt, :], xb[kt * P:(kt + 1) * P, :])
            w1_sb = const.tile([P, 2, nKD, HE], BF16)
            nc.scalar.dma_start(w1_sb[:], w1.rearrange("e (kd p) j -> p e kd j", p=P))
            w2_sb = const.tile([P, 2, nKH, O], BF16)
            nc.scalar.dma_start(w2_sb[:], w2.rearrange("e (kh p) j -> p e kh j", p=P))
            b1_sb = const.tile([P, 2 * nMH], F32)
            nc.scalar.dma_start(b1_sb[:], b1.rearrange("e (mh p) -> p (e mh)", p=P))

            # ---- HAM warmup: ~4us of dummy matmuls on uninitialized SBUF
            # during the initial DMA wait, so real matmuls start at 2.4GHz.
            # Results go to a scratch PSUM bank and are discarded.
            warm = const.tile([P, NF], BF16)
            nc.gpsimd.memset(warm[:], 0.0)
            ps_w = psp.tile([P, NF], F32, tag="ps", name="ps_warm")
            for i in range(8):
                nc.tensor.matmul(ps_w[:], warm[:, 0:P], warm[:],
                                 start=(i == 0), stop=(i == 7))

            y_tiles = []
            for ei in range(2):
                # ---- dispatch-mask tiles for this expert (SYNC ring) ----
                dm_t = []
                for kt in range(nKT):
                    t_ = dmp.tile([P, C], F8, tag="dm")
                    nc.sync.dma_start(t_[:], dm[ei, kt * P:(kt + 1) * P, :])
                    dm_t.append(t_)

                # ---- phase A: xdT[D, C] ----
                # kt-outer: all 8 PSUM banks accumulate in parallel, so each
                # dm tile is consumed once (at sustainable DMA rate) and
                # released immediately for the next expert's prefetch.
                xdt = inter.tile([P, nMD, C], BF16, tag="xdt")
                pss = [psp.tile([P, NF], F32, tag="ps", name=f"psa{i}")
                       for i in range(2 * nMD)]
                for kt in range(nKT):
                    for mc in range(nMD):
                        lhsT = x_sb[:, kt, mc * P:(mc + 1) * P]
                        nc.tensor.matmul(pss[2 * mc][:], lhsT, dm_t[kt][:, 0:NF],
                                         start=(kt == 0), stop=(kt == nKT - 1))
                        nc.tensor.matmul(pss[2 * mc + 1][:], lhsT, dm_t[kt][:, NF:C],
                                         start=(kt == 0), stop=(kt == nKT - 1))
                for ncc in range(2):
                    for mc in range(nMD):
                        nc.vector.tensor_copy(xdt[:, mc, ncc * NF:(ncc + 1) * NF],
                                              pss[2 * mc + ncc][:])

                # ---- phase B: hT[HE, C] = gelu(w1^T xdT + b1) ----
                # ncc-outer so phase C's first C-half unblocks after 4 gelus.
                # (kd-outer over 8 PSUM banks measured WORSE here: holding all
                # banks serializes the A->B transition.)
                ht = inter.tile([P, nMH, C], BF16, tag="ht")
                for ncc in range(2):
                    sl = slice(ncc * NF, (ncc + 1) * NF)
                    for mh in range(nMH):
                        ps0 = psp.tile([P, NF], F32, tag="ps")
                        for kd in range(nKD):
                            nc.tensor.matmul(ps0[:],
                                             w1_sb[:, ei, kd, mh * P:(mh + 1) * P],
                                             xdt[:, kd, sl],
                                             start=(kd == 0), stop=(kd == nKD - 1))
                        bia = b1_sb[:, ei * nMH + mh:ei * nMH + mh + 1]
                        nc.scalar.activation(ht[:, mh, sl], ps0[:], GELU, bias=bia)

                # ---- phase C: y[C, O] (stored fp8, DoubleRow plane layout:
                # row c = kp*256 + i*128 + p  ->  y_sb[p, kp, i, :]) ----
                y_sb = yp.tile([P, nCP, 2, O], F8, tag="y")
                for cc in range(nCC):
                    ps = psp.tile([P, NF], F32, tag="ps")
                    for kh in range(nKH):
                        nc.tensor.matmul(ps[:], ht[:, kh, cc * P:(cc + 1) * P],
                                         w2_sb[:, ei, kh, :],
                                         start=(kh == 0), stop=(kh == nKH - 1))
                    nc.vector.tensor_copy(y_sb[:, cc // 2, cc % 2, :], ps[:])
                y_tiles.append(y_sb)

            # ---- combine-mask tiles (fp8, [P, plane, T]): SYNC ring behind
            # the dm loads so they can't steal early HBM bandwidth ----
            cmt_t = {}
            for ei in range(2):
                for kp in range(nCP):
                    t_ = cmp_.tile([P, 2, T], F8, tag="cmt")
                    nc.sync.dma_start(
                        t_[:],
                        cmt[ei, kp * 2 * P:(kp + 1) * 2 * P, :]
                        .rearrange("(i p) t -> p i t", p=P))
                    cmt_t[(ei, kp)] = t_

            # ---- phase D: pout[T, O] = sum_e cmT_e^T y_e (fp8 DoubleRow) ----
            for mt in range(nMT):
                ps = psp.tile([P, NF], F32, tag="ps")
                idx = 0
                for ei in range(2):
                    for kp in range(nCP):
                        nc.tensor.matmul(ps[:],
                                         cmt_t[(ei, kp)][:, :, mt * P:(mt + 1) * P],
                                         y_tiles[ei][:, kp, :, :],
                                         start=(idx == 0), stop=(idx == 7),
                                         perf_mode=DR)
                        idx += 1
                ot = outp.tile([P, O], F32, tag="out")
                nc.vector.tensor_copy(ot[:], ps[:])
                nc.sync.dma_start(pout[mt * P:(mt + 1) * P, :], ot[:])

    nc.compile()
    return nc


def get_nc():
    global _NC
    if _NC is None:
        _NC = _build()
    return _NC


def make_in_maps(x, dispatch_mask, combine_array, w1, b1, w2):
    bf = ml_dtypes.bfloat16
    in_maps = []
    for m in range(8):
        b, g = m // 2, m % 2
        es = slice(2 * g, 2 * g + 2)
        dm_s = np.ascontiguousarray(
            np.transpose(dispatch_mask[b, :, es, :], (1, 0, 2))).astype(
                ml_dtypes.float8_e4m3)
        cmt_s = np.ascontiguousarray(
            np.transpose(combine_array[b, :, es, :], (1, 2, 0))).astype(
                ml_dtypes.float8_e4m3)
        in_maps.append({
            "xb": np.ascontiguousarray(x[b]).astype(bf),
            "dm": dm_s,
            "cmt": cmt_s,
            "w1": np.ascontiguousarray(w1[es]).astype(bf),
            "w2": np.ascontiguousarray(w2[es]).astype(bf),
            "b1s": np.ascontiguousarray(b1[es]).astype(np.float32),
        })
    return in_maps


def kernel(x, dispatch_mask, combine_array, w1, b1, w2, b2):
    nc = get_nc()
    x, dispatch_mask, combine_array, w1, b1, w2 = (
        np.asarray(a, dtype=np.float32)
        for a in (x, dispatch_mask, combine_array, w1, b1, w2))
    in_maps = make_in_maps(x, dispatch_mask, combine_array, w1, b1, w2)
    res = bass_utils.run_bass_kernel_spmd(nc, in_maps, core_ids=list(range(8)))
    b2f = np.asarray(b2, dtype=np.float32)
    out = np.empty((B, T, O), dtype=np.float32)
    for b in range(B):
        out[b] = res.results[2 * b]["pout"] + res.results[2 * b + 1]["pout"] + b2f
    return out



# revision 9
# speedup vs baseline: 1.3108x; 1.3108x over previous
"""ExpertsChooseMlp Trainium2 kernel.

Full inputs in, full output out. Sharding: 8 cores = 4 batches x 2 expert-pairs.
Core m handles batch b=m//2 and experts {2g, 2g+1}, g=m%2. Each core computes
pout[T,O] = sum_{e in pair} combine[b,:,e,:] @ mlp_e(dispatch[b,:,e,:]^T @ x[b]);
the host sums the two partials per batch, applies the w2 rank-1 correction
and adds b2.

Precision: every matmul operand in fp8-e4m3 (TRN variant: max +-240, values
above convert to Inf), accumulation in fp32 PSUM. All four contractions run
as fp8 DoubleRow matmuls (2 K-planes of 128 per pass) which issue at the
same ~215ns N=512 stream rate as bf16 -> 2x throughput, halving the matmul
count 512 -> 320. Host packs each K=256 block of the contraction dim as
[plane i][partition p] (row k = base + i*128 + p):
  xdT[D,C] = DRmm(lhsT=x[T2,i,D],   rhs=dm[T2,i,C])    (K=T,  8 passes)
  hT[HE,C] = DRmm(lhsT=w1[D2,i,HE], rhs=xdT[D2,i,C])   (K=D,  2 passes)
  y[C,O]   = DRmm(lhsT=hT[H2,i,C],  rhs=w2[H2,i,O])    (K=HE, 2 passes)
  pout[T,O]= DRmm(lhsT=cmT[C2,i,T], rhs=y[C2,i,O])     (K=C,  8 passes, +e)

fp8 error control: the output is dominated by a rank-1 "DC" component (the
masks have mean 0.5, gelu output has positive mean) which amplifies the
signal ~17x over generic per-element noise. Quantization error of x/w1/w2
rides that same DC path (their error column-sums are amplified); masks and
intermediates are not. Countermeasures, each killing the amplified term:
 - x: error-feedback quantization along T (quant-error prefix sums ~1 ulp).
 - w1: exact host bias fold b1 += mean_c(xd) @ (w1 - fp8(w1)); mean_c(xd)
   is host-computable from the quantized dm/x in O(T*(C+D)).
 - w2: device rank-1 correction. The gelu ACTIVATE's accum_out gives
   Hsum_h = sum_c ht[h,c] for free; 4 tiny matmuls/expert form
   t = Hsum @ (w2 - fp8(w2)); host adds outer(sum_c cm8, t)/C.
Measured in simulation: max rel err ~5e-3 (threshold 2e-2).
"""
import sys

sys.path.insert(0, "/opt/trn_rl_repo")

import numpy as np
import ml_dtypes

import concourse.bacc as bacc
import concourse.mybir as mybir
import concourse.tile as tile
from concourse import bass_utils

B, T, D, E, C, HE, O = 4, 2048, 512, 4, 1024, 512, 512
P = 128
nKT2 = T // 256   # 8  T DR-chunks (K=256 each)
nMD = D // P      # 4  D-chunks
nMH = HE // P     # 4  HE-chunks
nKD2 = D // 256   # 2  D DR-chunks
nCC = C // P      # 8  C-chunks
nKH2 = HE // 256  # 2  HE DR-chunks
nMT = T // P      # 16
NF = 512          # matmul free dim (one PSUM bank)

F32 = mybir.dt.float32
BF16 = mybir.dt.bfloat16
F8 = mybir.dt.float8e4
GELU = mybir.ActivationFunctionType.Gelu
COPY = mybir.ActivationFunctionType.Copy
DR = mybir.MatmulPerfMode.DoubleRow
nCP = nCC // 2    # 4  C pair-chunks

_NC = None


def _build():
    nc = bacc.Bacc("TRN2", target_bir_lowering=False, debug=False,
                   enable_asserts=False, num_devices=1)
    xq = nc.dram_tensor("xq", [P, nKT2, 2, D], F8, kind="ExternalInput").ap()
    dmq = nc.dram_tensor("dmq", [2, nKT2, P, 2, C], F8, kind="ExternalInput").ap()
    cmq = nc.dram_tensor("cmq", [2, nCP, P, 2, T], F8, kind="ExternalInput").ap()
    w1q = nc.dram_tensor("w1q", [P, 2, nKD2, 2, HE], F8, kind="ExternalInput").ap()
    w2q = nc.dram_tensor("w2q", [P, 2, nKH2, 2, O], F8, kind="ExternalInput").ap()
    dw2 = nc.dram_tensor("dw2", [P, 2, nMH, O], F32, kind="ExternalInput").ap()
    b1 = nc.dram_tensor("b1s", [2, HE], F32, kind="ExternalInput").ap()
    pout = nc.dram_tensor("pout", [T, O], BF16, kind="ExternalOutput").ap()
    tcorr = nc.dram_tensor("tcorr", [2, 2, O], F32, kind="ExternalOutput").ap()

    with tile.TileContext(nc) as tc:
        with (
            tc.tile_pool(name="const", bufs=1) as const,
            tc.tile_pool(name="dmp", bufs=16) as dmp,
            tc.tile_pool(name="cmp", bufs=8) as cmp_,
            tc.tile_pool(name="inter", bufs=1) as inter,
            tc.tile_pool(name="yp", bufs=2) as yp,
            tc.tile_pool(name="outp", bufs=4) as outp,
            tc.tile_pool(name="psum", bufs=8, space="PSUM") as psp,
        ):
            # ---- resident constants (ACT HWDGE ring) ----
            # x split per kt2-chunk so the first matmul isn't gated on a 1MB DMA
            x_sb = const.tile([P, nKT2, 2, D], F8)
            for kt in range(nKT2):
                nc.scalar.dma_start(x_sb[:, kt, :, :], xq[:, kt, :, :])
            b1_sb = const.tile([P, 2 * nMH], F32)
            nc.scalar.dma_start(b1_sb[:], b1.rearrange("e (mh p) -> p (e mh)", p=P))
            w1_sb = const.tile([P, 2, nKD2, 2, HE], F8)
            nc.scalar.dma_start(w1_sb[:], w1q[:])
            w2_sb = const.tile([P, 2, nKH2, 2, O], F8)
            nc.scalar.dma_start(w2_sb[:], w2q[:])
            dw2_sb = const.tile([P, 2, nMH, O], F32)
            nc.scalar.dma_start(dw2_sb[:], dw2[:])

            # ---- HAM warmup: dummy matmuls on a memset tile while the first
            # dm/x DMAs land, so real matmuls start closer to 2.4GHz.
            warm = const.tile([P, NF], BF16)
            nc.gpsimd.memset(warm[:], 0.0)
            ps_w = psp.tile([P, NF], F32, tag="ps", name="ps_warm")
            for i in range(3):
                nc.tensor.matmul(ps_w[:], warm[:, 0:P], warm[:],
                                 start=(i == 0), stop=(i == 2))

            y_tiles = []
            for ei in range(2):
                # ---- dispatch-mask tiles for this expert (SYNC ring) ----
                dm_t = []
                for kt in range(nKT2):
                    t_ = dmp.tile([P, 2, C], F8, tag="dm")
                    nc.sync.dma_start(t_[:], dmq[ei, kt])
                    dm_t.append(t_)

                # ---- phase A: xdT[D, C] (fp8 DR over K=T) ----
                # ncc-split: finish the 4 ncc=0 banks first so their PSUM->SBUF
                # evacuations overlap the ncc=1 accumulation pass and phase B
                # never starves on xdt copies. Copies split vector/scalar.
                xdt = inter.tile([P, nKD2, 2, C], F8, tag="xdt")
                pss = [psp.tile([P, NF], F32, tag="ps", name=f"psa{i}")
                       for i in range(2 * nMD)]
                for pn in range(2):
                    for kt in range(nKT2):
                        for mc in range(nMD):
                            nc.tensor.matmul(pss[2 * mc + pn][:],
                                             x_sb[:, kt, :, mc * P:(mc + 1) * P],
                                             dm_t[kt][:, :, pn * NF:(pn + 1) * NF],
                                             start=(kt == 0), stop=(kt == nKT2 - 1),
                                             perf_mode=DR)
                    # D-row d = mc*128 + p = (mc//2)*256 + (mc%2)*128 + p
                    for mc in range(nMD):
                        dst = xdt[:, mc // 2, mc % 2, pn * NF:(pn + 1) * NF]
                        if mc < 2:
                            nc.vector.tensor_copy(dst, pss[2 * mc + pn][:])
                        else:
                            nc.scalar.activation(dst, pss[2 * mc + pn][:], COPY)

                # ---- phase B: hT[HE, C] = gelu(w1^T xdT + b1) (fp8 DR) ----
                # ncc-outer so phase C's first C-half unblocks after 4 gelus.
                # accum_out collects Hsum_h = sum_c ht[h, c] for the w2 corr.
                ht = inter.tile([P, nKH2, 2, C], F8, tag="ht")
                hs = inter.tile([P, nMH, 2], F32, tag="hs")
                for ncc in range(2):
                    sl = slice(ncc * NF, (ncc + 1) * NF)
                    for mh in range(nMH):
                        ps0 = psp.tile([P, NF], F32, tag="ps")
                        for kd in range(nKD2):
                            nc.tensor.matmul(ps0[:],
                                             w1_sb[:, ei, kd, :, mh * P:(mh + 1) * P],
                                             xdt[:, kd, :, sl],
                                             start=(kd == 0), stop=(kd == nKD2 - 1),
                                             perf_mode=DR)
                        bia = b1_sb[:, ei * nMH + mh:ei * nMH + mh + 1]
                        nc.scalar.activation(ht[:, mh // 2, mh % 2, sl], ps0[:],
                                             GELU, bias=bia,
                                             accum_out=hs[:, mh, ncc:ncc + 1])

                # ---- phase C: y[C, O] (fp8 DR; DR plane layout:
                # row c = kp*256 + i*128 + p  ->  y_sb[p, kp, i, :]) ----
                y_sb = yp.tile([P, nCP, 2, O], F8, tag="y")
                for cc in range(nCC):
                    ps = psp.tile([P, NF], F32, tag="ps")
                    for kh in range(nKH2):
                        nc.tensor.matmul(ps[:],
                                         ht[:, kh, :, cc * P:(cc + 1) * P],
                                         w2_sb[:, ei, kh, :, :],
                                         start=(kh == 0), stop=(kh == nKH2 - 1),
                                         perf_mode=DR)
                    nc.vector.tensor_copy(y_sb[:, cc // 2, cc % 2, :], ps[:])
                y_tiles.append(y_sb)

                # ---- w2 rank-1 correction: t[2,O] = Hsum @ (w2 - fp8(w2)).
                # DMA rides the ACT ring (it's idle; a waiting descriptor there
                # can't block the mask loads on the SYNC ring).
                tps = psp.tile([2, NF], F32, tag="ps")
                for mh in range(nMH):
                    nc.tensor.matmul(tps[:], hs[:, mh, :], dw2_sb[:, ei, mh, :],
                                     start=(mh == 0), stop=(mh == nMH - 1))
                tc_sb = outp.tile([2, O], F32, tag="tc")
                nc.vector.tensor_copy(tc_sb[:], tps[:])
                nc.scalar.dma_start(tcorr[ei], tc_sb[:])

            # ---- combine-mask tiles (fp8, [P, plane, T]): SYNC ring behind
            # the dm loads so they can't steal early HBM bandwidth ----
            cmt_t = {}
            for ei in range(2):
                for kp in range(nCP):
                    t_ = cmp_.tile([P, 2, T], F8, tag="cmt")
                    nc.sync.dma_start(t_[:], cmq[ei, kp])
                    cmt_t[(ei, kp)] = t_

            # ---- phase D: pout[T, O] = sum_e cmT_e^T y_e (fp8 DR) ----
            for mt in range(nMT):
                ps = psp.tile([P, NF], F32, tag="ps")
                idx = 0
                for ei in range(2):
                    for kp in range(nCP):
                        nc.tensor.matmul(ps[:],
                                         cmt_t[(ei, kp)][:, :, mt * P:(mt + 1) * P],
                                         y_tiles[ei][:, kp, :, :],
                                         start=(idx == 0), stop=(idx == 7),
                                         perf_mode=DR)
                        idx += 1
                ot = outp.tile([P, O], BF16, tag="out")
                nc.vector.tensor_copy(ot[:], ps[:])
                nc.sync.dma_start(pout[mt * P:(mt + 1) * P, :], ot[:])

    nc.compile()
    return nc


def get_nc():
    global _NC
    if _NC is None:
        _NC = _build()
    return _NC


_F8 = ml_dtypes.float8_e4m3


def _qef(a):
    """fp8 quantization with error feedback along axis 0 (keeps running
    column sums of the quantization error bounded by ~1 ulp)."""
    out = np.empty(a.shape, _F8)
    carry = np.zeros(a.shape[1:], np.float32)
    for t in range(a.shape[0]):
        v = a[t] + carry
        q = v.astype(_F8)
        out[t] = q
        carry = v - q.astype(np.float32)
    return out


def make_in_maps(x, dispatch_mask, combine_array, w1, b1, w2):
    in_maps = []
    meta = []
    x8 = {}
    for b in range(B):
        x8[b] = _qef(x[b])                       # [T, D] fp8, EF along T
    for m in range(8):
        b, g = m // 2, m % 2
        es = slice(2 * g, 2 * g + 2)
        x8f = x8[b].astype(np.float32)
        xs = np.ascontiguousarray(
            x8[b].reshape(nKT2, 2, P, D).transpose(2, 0, 1, 3))
        # dm: [e, kt2, p, i, c], row t = kt2*256 + i*128 + p
        dm_s = np.transpose(dispatch_mask[b, :, es, :], (1, 0, 2)).astype(_F8)
        dm_q = np.ascontiguousarray(
            dm_s.reshape(2, nKT2, 2, P, C).transpose(0, 1, 3, 2, 4))
        # cmT: [e, kp, p, i, t], row c = kp*256 + i*128 + p
        cm_s = np.transpose(combine_array[b, :, es, :], (1, 2, 0)).astype(_F8)
        cm_q = np.ascontiguousarray(
            cm_s.reshape(2, nCP, 2, P, T).transpose(0, 1, 3, 2, 4))
        w1_8 = w1[es].astype(_F8)                # [2, D, HE]
        w2_8 = w2[es].astype(_F8)                # [2, HE, O]
        # w1 bias fold: b1' = b1 + mean_c(xd) @ (w1 - w18),
        # mean_c(xd)_d = sum_t mean_c(dm8[t,:]) * x8[t,d]
        b1c = np.empty((2, HE), np.float32)
        for e in range(2):
            rm = dm_s[e].astype(np.float32).mean(axis=1)      # [T]
            xbar = rm @ x8f                                   # [D]
            b1c[e] = b1[es][e] + xbar @ (w1[es][e] - w1_8[e].astype(np.float32))
        dw2_s = (w2[es] - w2_8.astype(np.float32))            # [2, HE, O]
        dw2_q = np.ascontiguousarray(
            dw2_s.reshape(2, nMH, P, O).transpose(2, 0, 1, 3)).astype(np.float32)
        # host-side combine weights for the w2 correction
        cmsum = cm_s.astype(np.float32).sum(axis=1)           # [2, T]
        meta.append(cmsum)
        in_maps.append({
            "xq": xs,
            "dmq": dm_q,
            "cmq": cm_q,
            "w1q": np.ascontiguousarray(
                w1_8.reshape(2, nKD2, 2, P, HE).transpose(3, 0, 1, 2, 4)),
            "w2q": np.ascontiguousarray(
                w2_8.reshape(2, nKH2, 2, P, O).transpose(3, 0, 1, 2, 4)),
            "dw2": dw2_q,
            "b1s": np.ascontiguousarray(b1c),
        })
    return in_maps, meta


def kernel(x, dispatch_mask, combine_array, w1, b1, w2, b2):
    nc = get_nc()
    x, dispatch_mask, combine_array, w1, b1, w2 = (
        np.asarray(a, dtype=np.float32)
        for a in (x, dispatch_mask, combine_array, w1, b1, w2))
    in_maps, meta = make_in_maps(x, dispatch_mask, combine_array, w1, b1, w2)
    res = bass_utils.run_bass_kernel_spmd(nc, in_maps, core_ids=list(range(8)))
    b2f = np.asarray(b2, dtype=np.float32)
    out = np.empty((B, T, O), dtype=np.float32)
    for b in range(B):
        acc = np.zeros((T, O), np.float32)
        for g in range(2):
            m = 2 * b + g
            acc += res.results[m]["pout"].astype(np.float32)
            tc = res.results[m]["tcorr"].astype(np.float32)   # [2, 2, O]
            cmsum = meta[m]                                   # [2, T]
            for e in range(2):
                t_full = (tc[e, 0] + tc[e, 1]) * (1.0 / C)    # [O]
                acc += np.outer(cmsum[e], t_full)
        out[b] = acc + b2f
    return out


# revision 15
# speedup vs baseline: 1.4025x; 1.0699x over previous
"""ExpertsChooseMlp Trainium2 kernel.

Full inputs in, full output out. Sharding: 8 cores = 4 batches x 2 expert-pairs.
Core m handles batch b=m//2 and experts {2g, 2g+1}, g=m%2. Each core computes
pout[T,O] = sum_{e in pair} combine[b,:,e,:] @ mlp_e(dispatch[b,:,e,:]^T @ x[b]);
the host sums the two partials per batch, applies the w2 rank-1 correction
and adds b2.

Precision: every matmul operand in fp8-e4m3 (TRN variant: max +-240, values
above convert to Inf), accumulation in fp32 PSUM. All four contractions run
as fp8 DoubleRow matmuls (2 K-planes of 128 per pass) which issue at the
same ~215ns N=512 stream rate as bf16 -> 2x throughput, halving the matmul
count 512 -> 320. Host packs each K=256 block of the contraction dim as
[plane i][partition p] (row k = base + i*128 + p):
  xdT[D,C] = DRmm(lhsT=x[T2,i,D],   rhs=dm[T2,i,C])    (K=T,  8 passes)
  hT[HE,C] = DRmm(lhsT=w1[D2,i,HE], rhs=xdT[D2,i,C])   (K=D,  2 passes)
  y[C,O]   = DRmm(lhsT=hT[H2,i,C],  rhs=w2[H2,i,O])    (K=HE, 2 passes)
  pout[T,O]= DRmm(lhsT=cmT[C2,i,T], rhs=y[C2,i,O])     (K=C,  8 passes, +e)

fp8 error control: the output is dominated by a rank-1 "DC" component (the
masks have mean 0.5, gelu output has positive mean) which amplifies the
signal ~17x over generic per-element noise. Quantization error of x/w1/w2
rides that same DC path (their error column-sums are amplified); masks and
intermediates are not. Countermeasures, each killing the amplified term:
 - x: error-feedback quantization along T (quant-error prefix sums ~1 ulp).
 - w1: exact host bias fold b1 += mean_c(xd) @ (w1 - fp8(w1)); mean_c(xd)
   is host-computable from the quantized dm/x in O(T*(C+D)).
 - w2: device rank-1 correction. The gelu ACTIVATE's accum_out gives
   Hsum_h = sum_c ht[h,c] for free; 4 tiny matmuls/expert form
   t = Hsum @ (w2 - fp8(w2)); host adds outer(sum_c cm8, t)/C.
Measured in simulation: max rel err ~5e-3 (threshold 2e-2).
"""
import sys

sys.path.insert(0, "/opt/trn_rl_repo")

import numpy as np
import ml_dtypes

import concourse.bacc as bacc
import concourse.mybir as mybir
import concourse.tile as tile
from concourse import bass_utils

B, T, D, E, C, HE, O = 4, 2048, 512, 4, 1024, 512, 512
P = 128
nKT2 = T // 256   # 8  T DR-chunks (K=256 each)
nMD = D // P      # 4  D-chunks
nMH = HE // P     # 4  HE-chunks
nKD2 = D // 256   # 2  D DR-chunks
nCC = C // P      # 8  C-chunks
nKH2 = HE // 256  # 2  HE DR-chunks
nMT = T // P      # 16
NF = 512          # matmul free dim (one PSUM bank)

F32 = mybir.dt.float32
BF16 = mybir.dt.bfloat16
F8 = mybir.dt.float8e4
GELU = mybir.ActivationFunctionType.Gelu
COPY = mybir.ActivationFunctionType.Copy
DR = mybir.MatmulPerfMode.DoubleRow
nCP = nCC // 2    # 4  C pair-chunks

_NC = None


def _build():
    nc = bacc.Bacc("TRN2", target_bir_lowering=False, debug=False,
                   enable_asserts=False, num_devices=1)
    xq = nc.dram_tensor("xq", [P, nKT2, 2, D], F8, kind="ExternalInput").ap()
    dmq = nc.dram_tensor("dmq", [2, nKT2, P, 2, C], F8, kind="ExternalInput").ap()
    cmq = nc.dram_tensor("cmq", [2, nCP, P, 2, T], F8, kind="ExternalInput").ap()
    w1q = nc.dram_tensor("w1q", [P, 2, nKD2, 2, HE], F8, kind="ExternalInput").ap()
    w2q = nc.dram_tensor("w2q", [P, 2, nKH2, 2, O], F8, kind="ExternalInput").ap()
    dw2 = nc.dram_tensor("dw2", [P, 2, nMH, O], BF16, kind="ExternalInput").ap()
    b1 = nc.dram_tensor("b1s", [2, HE], F32, kind="ExternalInput").ap()
    pout = nc.dram_tensor("pout", [T, O], BF16, kind="ExternalOutput").ap()
    tcorr = nc.dram_tensor("tcorr", [2, 2, O], F32, kind="ExternalOutput").ap()

    with tile.TileContext(nc) as tc:
        with (
            tc.tile_pool(name="const", bufs=1) as const,
            tc.tile_pool(name="dmp", bufs=16) as dmp,
            tc.tile_pool(name="cmp", bufs=8) as cmp_,
            tc.tile_pool(name="inter", bufs=1) as inter,
            tc.tile_pool(name="yp", bufs=2) as yp,
            tc.tile_pool(name="outp", bufs=4) as outp,
            tc.tile_pool(name="psum", bufs=8, space="PSUM") as psp,
        ):
            # ---- resident constants (ACT HWDGE ring) ----
            # x split per kt2-chunk so the first matmul isn't gated on a 1MB DMA
            x_sb = const.tile([P, nKT2, 2, D], F8)
            for kt in range(nKT2):
                nc.scalar.dma_start(x_sb[:, kt, :, :], xq[:, kt, :, :])
            b1_sb = const.tile([P, 2 * nMH], F32)
            nc.scalar.dma_start(b1_sb[:], b1.rearrange("e (mh p) -> p (e mh)", p=P))
            w1_sb = const.tile([P, 2, nKD2, 2, HE], F8)
            nc.scalar.dma_start(w1_sb[:], w1q[:])
            w2_sb = const.tile([P, 2, nKH2, 2, O], F8)
            nc.scalar.dma_start(w2_sb[:], w2q[:])
            dw2_sb = const.tile([P, 2, nMH, O], BF16)
            nc.scalar.dma_start(dw2_sb[:], dw2[:])

            # ---- HAM warmup: dummy matmuls on a memset tile while the first
            # dm/x DMAs land, so real matmuls start closer to 2.4GHz.
            warm = const.tile([P, NF], BF16)
            nc.gpsimd.memset(warm[:], 0.0)
            # 8 warm matmuls bridge the ~3.3us gap until the first DMA batch's
            # completion semaphores release the first real matmul (~10.6us),
            # so the HAM clock gate is already at 8/8 when real work starts.
            ps_w = psp.tile([P, NF], F32, tag="ps", name="ps_warm")
            for i in range(8):
                nc.tensor.matmul(ps_w[:], warm[:, 0:P], warm[:],
                                 start=(i == 0), stop=(i == 7))

            y_tiles = []
            for ei in range(2):
                # ---- dispatch-mask tiles for this expert (SYNC ring) ----
                dm_t = []
                for kt in range(nKT2):
                    t_ = dmp.tile([P, 2, C], F8, tag="dm")
                    nc.sync.dma_start(t_[:], dmq[ei, kt])
                    dm_t.append(t_)

                # ---- phase A: xdT[D, C] (fp8 DR over K=T) ----
                # ncc-split: finish the 4 ncc=0 banks first so their PSUM->SBUF
                # evacuations overlap the ncc=1 accumulation pass and phase B
                # never starves on xdt copies. Copies split vector/scalar.
                xdt = inter.tile([P, nKD2, 2, C], F8, tag="xdt")
                pss = [psp.tile([P, NF], F32, tag="ps", name=f"psa{i}")
                       for i in range(2 * nMD)]
                for pn in range(2):
                    for kt in range(nKT2):
                        for mc in range(nMD):
                            nc.tensor.matmul(pss[2 * mc + pn][:],
                                             x_sb[:, kt, :, mc * P:(mc + 1) * P],
                                             dm_t[kt][:, :, pn * NF:(pn + 1) * NF],
                                             start=(kt == 0), stop=(kt == nKT2 - 1),
                                             perf_mode=DR)
                    # D-row d = mc*128 + p = (mc//2)*256 + (mc%2)*128 + p.
                    # All copies on DVE: ScalarE must stay GELU-only or it
                    # reloads the activation table (1.3us) at every switch.
                    for mc in range(nMD):
                        nc.vector.tensor_copy(
                            xdt[:, mc // 2, mc % 2, pn * NF:(pn + 1) * NF],
                            pss[2 * mc + pn][:])

                # ---- phase B: hT[HE, C] = gelu(w1^T xdT + b1) (fp8 DR) ----
                # ncc-outer so phase C's first C-half unblocks after 4 gelus.
                # accum_out collects Hsum_h = sum_c ht[h, c] for the w2 corr.
                ht = inter.tile([P, nKH2, 2, C], F8, tag="ht")
                hs = inter.tile([P, nMH, 2], BF16, tag="hs")
                for ncc in range(2):
                    sl = slice(ncc * NF, (ncc + 1) * NF)
                    for mh in range(nMH):
                        ps0 = psp.tile([P, NF], F32, tag="ps")
                        for kd in range(nKD2):
                            nc.tensor.matmul(ps0[:],
                                             w1_sb[:, ei, kd, :, mh * P:(mh + 1) * P],
                                             xdt[:, kd, :, sl],
                                             start=(kd == 0), stop=(kd == nKD2 - 1),
                                             perf_mode=DR)
                        bia = b1_sb[:, ei * nMH + mh:ei * nMH + mh + 1]
                        # bf16 accum_out: it feeds the rank-1 correction whose
                        # own magnitude is ~2% of the output, so bf16's 0.4%
                        # is harmless — and the t-matmul stays single-pass
                        # (an fp32 matmul runs LOW_HIGH double-pass on PE).
                        with nc.allow_low_precision(reason="w2-corr accum"):
                            nc.scalar.activation(ht[:, mh // 2, mh % 2, sl],
                                                 ps0[:], GELU, bias=bia,
                                                 accum_out=hs[:, mh, ncc:ncc + 1])

                # ---- phase C: y[C, O] (fp8 DR; DR plane layout:
                # row c = kp*256 + i*128 + p  ->  y_sb[p, kp, i, :]) ----
                y_sb = yp.tile([P, nCP, 2, O], F8, tag="y")
                for cc in range(nCC):
                    ps = psp.tile([P, NF], F32, tag="ps")
                    for kh in range(nKH2):
                        nc.tensor.matmul(ps[:],
                                         ht[:, kh, :, cc * P:(cc + 1) * P],
                                         w2_sb[:, ei, kh, :, :],
                                         start=(kh == 0), stop=(kh == nKH2 - 1),
                                         perf_mode=DR)
                    nc.vector.tensor_copy(y_sb[:, cc // 2, cc % 2, :], ps[:])
                y_tiles.append(y_sb)

                # ---- w2 rank-1 correction: t[2,O] = Hsum @ (w2 - fp8(w2)).
                # DMA rides the ACT ring (it's idle; a waiting descriptor there
                # can't block the mask loads on the SYNC ring).
                tps = psp.tile([2, NF], F32, tag="ps")
                for mh in range(nMH):
                    nc.tensor.matmul(tps[:], hs[:, mh, :], dw2_sb[:, ei, mh, :],
                                     start=(mh == 0), stop=(mh == nMH - 1))
                tc_sb = outp.tile([2, O], F32, tag="tc")
                nc.vector.tensor_copy(tc_sb[:], tps[:])
                nc.scalar.dma_start(tcorr[ei], tc_sb[:])

            # ---- combine-mask tiles (fp8, [P, plane, T]): SYNC ring behind
            # the dm loads so they can't steal early HBM bandwidth ----
            cmt_t = {}
            for ei in range(2):
                for kp in range(nCP):
                    t_ = cmp_.tile([P, 2, T], F8, tag="cmt")
                    nc.sync.dma_start(t_[:], cmq[ei, kp])
                    cmt_t[(ei, kp)] = t_

            # ---- phase D: pout[T, O] = sum_e cmT_e^T y_e (fp8 DR) ----
            for mt in range(nMT):
                ps = psp.tile([P, NF], F32, tag="ps")
                idx = 0
                for ei in range(2):
                    for kp in range(nCP):
                        nc.tensor.matmul(ps[:],
                                         cmt_t[(ei, kp)][:, :, mt * P:(mt + 1) * P],
                                         y_tiles[ei][:, kp, :, :],
                                         start=(idx == 0), stop=(idx == 7),
                                         perf_mode=DR)
                        idx += 1
                ot = outp.tile([P, O], BF16, tag="out")
                nc.vector.tensor_copy(ot[:], ps[:])
                nc.sync.dma_start(pout[mt * P:(mt + 1) * P, :], ot[:])

    nc.compile()
    return nc


def get_nc():
    global _NC
    if _NC is None:
        _NC = _build()
    return _NC


_F8 = ml_dtypes.float8_e4m3


def _qef(a):
    """fp8 quantization with error feedback along axis 0 (keeps running
    column sums of the quantization error bounded by ~1 ulp)."""
    out = np.empty(a.shape, _F8)
    carry = np.zeros(a.shape[1:], np.float32)
    for t in range(a.shape[0]):
        v = a[t] + carry
        q = v.astype(_F8)
        out[t] = q
        carry = v - q.astype(np.float32)
    return out


def make_in_maps(x, dispatch_mask, combine_array, w1, b1, w2):
    in_maps = []
    meta = []
    x8 = {}
    for b in range(B):
        x8[b] = _qef(x[b])                       # [T, D] fp8, EF along T
    for m in range(8):
        b, g = m // 2, m % 2
        es = slice(2 * g, 2 * g + 2)
        x8f = x8[b].astype(np.float32)
        xs = np.ascontiguousarray(
            x8[b].reshape(nKT2, 2, P, D).transpose(2, 0, 1, 3))
        # dm: [e, kt2, p, i, c], row t = kt2*256 + i*128 + p
        dm_s = np.transpose(dispatch_mask[b, :, es, :], (1, 0, 2)).astype(_F8)
        dm_q = np.ascontiguousarray(
            dm_s.reshape(2, nKT2, 2, P, C).transpose(0, 1, 3, 2, 4))
        # cmT: [e, kp, p, i, t], row c = kp*256 + i*128 + p
        cm_s = np.transpose(combine_array[b, :, es, :], (1, 2, 0)).astype(_F8)
        cm_q = np.ascontiguousarray(
            cm_s.reshape(2, nCP, 2, P, T).transpose(0, 1, 3, 2, 4))
        w1_8 = w1[es].astype(_F8)                # [2, D, HE]
        w2_8 = w2[es].astype(_F8)                # [2, HE, O]
        # w1 bias fold: b1' = b1 + mean_c(xd) @ (w1 - w18),
        # mean_c(xd)_d = sum_t mean_c(dm8[t,:]) * x8[t,d]
        b1c = np.empty((2, HE), np.float32)
        for e in range(2):
            rm = dm_s[e].astype(np.float32).mean(axis=1)      # [T]
            xbar = rm @ x8f                                   # [D]
            b1c[e] = b1[es][e] + xbar @ (w1[es][e] - w1_8[e].astype(np.float32))
        dw2_s = (w2[es] - w2_8.astype(np.float32))            # [2, HE, O]
        dw2_q = np.ascontiguousarray(
            dw2_s.reshape(2, nMH, P, O).transpose(2, 0, 1, 3)
        ).astype(ml_dtypes.bfloat16)
        # host-side combine weights for the w2 correction
        cmsum = cm_s.astype(np.float32).sum(axis=1)           # [2, T]
        meta.append(cmsum)
        in_maps.append({
            "xq": xs,
            "dmq": dm_q,
            "cmq": cm_q,
            "w1q": np.ascontiguousarray(
                w1_8.reshape(2, nKD2, 2, P, HE).transpose(3, 0, 1, 2, 4)),
            "w2q": np.ascontiguousarray(
                w2_8.reshape(2, nKH2, 2, P, O).transpose(3, 0, 1, 2, 4)),
            "dw2": dw2_q,
            "b1s": np.ascontiguousarray(b1c),
        })
    return in_maps, meta


def kernel(x, dispatch_mask, combine_array, w1, b1, w2, b2):
    nc = get_nc()
    x, dispatch_mask, combine_array, w1, b1, w2 = (
        np.asarray(a, dtype=np.float32)
        for a in (x, dispatch_mask, combine_array, w1, b1, w2))
    in_maps, meta = make_in_maps(x, dispatch_mask, combine_array, w1, b1, w2)
    res = bass_utils.run_bass_kernel_spmd(nc, in_maps, core_ids=list(range(8)))
    b2f = np.asarray(b2, dtype=np.float32)
    out = np.empty((B, T, O), dtype=np.float32)
    for b in range(B):
        acc = np.zeros((T, O), np.float32)
        for g in range(2):
            m = 2 * b + g
            acc += res.results[m]["pout"].astype(np.float32)
            tc = res.results[m]["tcorr"].astype(np.float32)   # [2, 2, O]
            cmsum = meta[m]                                   # [2, T]
            for e in range(2):
                t_full = (tc[e, 0] + tc[e, 1]) * (1.0 / C)    # [O]
                acc += np.outer(cmsum[e], t_full)
        out[b] = acc + b2f
    return out
